# revision 14
# baseline (speedup 1.0000x reference)
"""Multi-head causal attention kernel for Trainium2 (8 NeuronCores, batch-parallel).

Problem: B=8, Tx=Tz=1024, Dx=Dz=1024, Datt=Dmid=64, H=16, Dout=1024, causal mask.
Sharding: batch dim across the 8 cores (one batch element per core) - weights
replicated, no collectives needed.

Per-core dataflow (matmul operands in fp16, all accumulation in fp32 PSUM):
  xT,zT : host-transposed activations [d, t]
  V     = zT.T @ Wv_cat + bv    -> [z, h*65+e] with a ones column per head
                                   (bias via K=1 matmul; ones col via DVE fill)
  per head-pair P (2 heads at partition offsets 0/64):
    QT_P = Wq_cat[:,P].T @ xT + bq  -> [128(he), 512(x)] per x-chunk
    KT_P = Wk_cat[:,P].T @ zT + bk  -> [128(he), 512(z)] per z-chunk
    S^T  = lhsT=KT[64,128] x rhs=QT[64,512] -> 2-bank psum [z, 2*x] (row-packed)
    A^T  = exp(S^T/8) both heads in one ACT op, causal-trimmed, diag masked (DVE)
    yT   = V_aug.T @ A^T -> psum [65, x]: rows 0..63 = y^T, row 64 = sumexp
    norm: 1/sumexp (DVE approx) -> partition_broadcast (GPSIMD) -> mul (DVE)
          -> yT tiles split per (x-chunk, pair) for fine-grained consumption
  out  = yT_cat.T @ Wp + bp, split into pass A (pairs 0..6, accumulated into
         SBUF fp32 while pair 7 attention runs) + pass B (pair 7's single
         matmul + fused add) so the post-last-norm tail is short

Perf notes (vs first working version):
  - all weight staging is partition-major so every DMA line is >=1KB
    contiguous (DMA engines are descriptor-rate-bound at ~42ns/descriptor)
  - V phase consumes the resident zT k-tiles directly (no separate z staging)
  - warm-up matmuls on scratch SBUF during the initial DMA wait keep the PE
    HAM clock at 2.4GHz for the first real matmuls
"""
import sys
import types

sys.path.insert(0, "/opt/trn_rl_repo")

# bass_utils imports antenv.axon_hooks when tracing is requested (e.g. via a
# BASS_TRACE env var); that module doesn't exist in this image. Provide a
# no-op stub so tracing degrades gracefully instead of crashing. A test
# harness can pre-register a real hook module before importing this file.
if "antenv.axon_hooks" not in sys.modules:
    _m = types.ModuleType("antenv.axon_hooks")
    _m.get_axon_ntff_profile_hook = lambda: None
    sys.modules["antenv.axon_hooks"] = _m

import numpy as np

import concourse.bacc as bacc
import concourse.mybir as mybir
import concourse.tile as tile
from concourse.bass_utils import run_bass_kernel_spmd

F32 = mybir.dt.float32
FP16 = mybir.dt.float16

B, T, D, E, H = 8, 1024, 1024, 64, 16
NK = D // 128          # 8 contraction tiles
NP = H // 2            # 8 head pairs
NJ = T // 128          # 8 z tiles
NC = T // 512          # 2 x chunks
SCALE = 0.125          # 1/sqrt(64)
NWARM = 8              # PE warm-up matmuls during DMA lead-in


def build_program():
    nc = bacc.Bacc("TRN2", target_bir_lowering=False, debug=False)

    xT_d = nc.dram_tensor("xT", [D, T], FP16, kind="ExternalInput")
    zT_d = nc.dram_tensor("zT", [D, T], FP16, kind="ExternalInput")
    # partition-major stagings: [128, ...] with per-partition-contiguous lines
    zTzb_d = nc.dram_tensor("zTzb", [128, NJ * NK * 128], FP16, kind="ExternalInput")
    wq_d = nc.dram_tensor("wq", [128, NP * NK * 128], FP16, kind="ExternalInput")
    wk_d = nc.dram_tensor("wk", [128, NP * NK * 128], FP16, kind="ExternalInput")
    wv_d = nc.dram_tensor("wv", [128, 2 * NK * 512], FP16, kind="ExternalInput")
    wp_d = nc.dram_tensor("wp", [128, 2 * NK * 512], FP16, kind="ExternalInput")
    bqk_d = nc.dram_tensor("bqk", [128, 16], F32, kind="ExternalInput")
    bvb_d = nc.dram_tensor("bvb", [128, H * E], FP16, kind="ExternalInput")
    bpb_d = nc.dram_tensor("bpb", [128, H * E], F32, kind="ExternalInput")
    maskt_d = nc.dram_tensor("maskt", [128, 256], FP16, kind="ExternalInput")
    out_d = nc.dram_tensor("out", [128, NJ, T], FP16, kind="ExternalOutput")

    Exp = mybir.ActivationFunctionType.Exp

    with tile.TileContext(nc) as tc:
        with (
            tc.tile_pool(name="big", bufs=1) as big,
            tc.tile_pool(name="wf", bufs=2) as wf,
            tc.tile_pool(name="wb", bufs=6) as wb,
            tc.tile_pool(name="qk", bufs=12) as qk,
            tc.tile_pool(name="apool", bufs=10) as apool,
            tc.tile_pool(name="norm", bufs=6) as norm,
            tc.tile_pool(name="opool", bufs=3) as opool,
            tc.tile_pool(name="cst", bufs=1) as cst,
            tc.tile_pool(name="mps", bufs=2, space="PSUM") as mps,
            tc.tile_pool(name="sps", bufs=2, space="PSUM") as sps,
            tc.tile_pool(name="yps", bufs=2, space="PSUM") as yps,
        ):
            # ---- constants + warm-up scratch ----
            bqk_t = cst.tile([128, 16], F32)
            bvb_t = cst.tile([128, H * E], FP16)
            bpb_t = cst.tile([128, H * E], F32)
            maskt_t = cst.tile([128, 256], FP16)
            onesf_t = cst.tile([128, 16], FP16)
            scratch_t = cst.tile([128, 512], FP16)
            nc.gpsimd.memset(onesf_t[:], 1.0)
            nc.gpsimd.memset(scratch_t[:], 0.0)
            # warm-up matmuls: no DMA deps, keep the PE busy (and the HAM
            # clock ramping to 2.4GHz) while the first input tiles stream in
            warm_ps = mps.tile([128, 512], F32, tag="mps", name="warm")
            for _ in range(NWARM):
                nc.tensor.matmul(warm_ps[:], scratch_t[:, 0:128], scratch_t[:],
                                 start=True, stop=True, skip_group_check=True)

            # ---- resident activations ----
            xT_t = [big.tile([128, T], FP16, tag="xTk", bufs=NK, name=f"xT{k}")
                    for k in range(NK)]
            zT_t = [big.tile([128, T], FP16, tag="zTk", bufs=NK, name=f"zT{k}")
                    for k in range(NK)]
            V_t = big.tile([128, NJ, H * 65], FP16, tag="V")
            # yT split per (x-chunk, pair): out-proj pass A consumes pairs as
            # they complete instead of waiting for the whole P loop
            yTt = [[big.tile([128, 512], FP16, tag="yTt", bufs=NC * NP,
                             name=f"yT{c}_{P}") for P in range(NP)]
                   for c in range(NC)]
            zTzb_r = zTzb_d.ap().rearrange("p (zb k c) -> p zb k c", k=NK, c=128)
            zTzb_t = [big.tile([128, NK, 128], FP16, tag="zTzb", bufs=NJ,
                               name=f"zTzb{zb}") for zb in range(NJ)]
            wq_r = wq_d.ap().rearrange("p (P k c) -> p P k c", k=NK, c=128)
            wk_r = wk_d.ap().rearrange("p (P k c) -> p P k c", k=NK, c=128)
            wv_r = wv_d.ap().rearrange("p (vc k c) -> p vc k c", k=NK, c=512)
            wp_r = wp_d.ap().rearrange("p (dc ht c) -> p dc ht c", ht=NK, c=512)
            xT_r = xT_d.ap().rearrange("(k p) t -> p k t", p=128)
            zT_r = zT_d.ap().rearrange("(k p) t -> p k t", p=128)
            # DMA order = need order: V phase first (zT k-tiles + wv halves +
            # bvb), attention consts, wv second half, xT (Q proj), bpb
            wvh0 = [wf.tile([128, 512], FP16, tag="wv0", bufs=NK, name=f"wvh0_{k}")
                    for k in range(NK)]
            nc.sync.dma_start(zTzb_t[0][:], zTzb_r[:, 0, :, :])
            for k in range(NK):
                nc.sync.dma_start(wvh0[k][:], wv_r[:, 0, k, :])
                if k < 3:
                    nc.sync.dma_start(zTzb_t[k + 1][:], zTzb_r[:, k + 1, :, :])
            nc.sync.dma_start(bvb_t[:], bvb_d.ap())
            for zb in range(4, NJ):
                nc.sync.dma_start(zTzb_t[zb][:], zTzb_r[:, zb, :, :])
            nc.sync.dma_start(bqk_t[:], bqk_d.ap())
            nc.sync.dma_start(maskt_t[:], maskt_d.ap())
            wvh1 = [wf.tile([128, 512], FP16, tag="wv1", bufs=NK, name=f"wvh1_{k}")
                    for k in range(NK)]
            for k in range(NK):
                nc.sync.dma_start(wvh1[k][:], wv_r[:, 1, k, :])
            for k in range(NK):
                nc.sync.dma_start(xT_t[k][:], xT_r[:, k, :])
            for k in range(NK):
                nc.sync.dma_start(zT_t[k][:], zT_r[:, k, :])
            nc.sync.dma_start(bpb_t[:], bpb_d.ap())

            # ---- V phase: V[z, he] = zT.T @ Wv + bv (65-col/head layout) ----
            for vc in range(2):
                for zb in range(NJ):
                    # alternate between the mps and (idle-until-P-loop) sps
                    # pools: 4-deep psum pipelining hides eviction latency
                    vpool = mps if zb % 2 == 0 else sps
                    ps = vpool.tile([128, 512], F32,
                                    tag="mps" if zb % 2 == 0 else "sps")
                    for k in range(NK):
                        rhs = wvh0[k][:] if vc == 0 else wvh1[k][:]
                        nc.tensor.matmul(
                            ps[:], zTzb_t[zb][:, k, :], rhs,
                            start=(k == 0), stop=(k == NK - 1),
                        )
                    dst = V_t[:, zb, vc * 520:(vc + 1) * 520].rearrange(
                        "p (h c) -> p h c", c=65)[:, :, 0:64]
                    nc.vector.tensor_add(
                        dst, ps[:].rearrange("p (h c) -> p h c", c=64),
                        bvb_t[:, vc * 512:(vc + 1) * 512].rearrange("p (h c) -> p h c", c=64))
            for zb in range(NJ):
                ones_dst = V_t[:, zb, :].rearrange("p (h c) -> p h c", c=65)[:, :, 64:65]
                nc.vector.tensor_copy(ones_dst, onesf_t[:].rearrange("p (h c) -> p h c", c=1))

            wph = []

            # ---- head-pair loop ----
            for P in range(NP):
                wqP = wb.tile([128, NK, 128], FP16, tag="wb")
                nc.sync.dma_start(wqP[:], wq_r[:, P, :, :])
                wkP = wb.tile([128, NK, 128], FP16, tag="wb")
                nc.sync.dma_start(wkP[:], wk_r[:, P, :, :])
                if P == 1:
                    # prefetch Wp halves now: P0/P1 weight DMAs are already
                    # queued, and the wf ring slot waits for V-phase release
                    for dc in range(2):
                        w = wf.tile([128, NK, 512], FP16, tag="wf", name=f"wph{dc}")
                        nc.sync.dma_start(w[:], wp_r[:, dc, :, :])
                        wph.append(w)

                QT = []
                for c in range(NC):
                    qt = qk.tile([128, 512], FP16, tag="qk", name=f"QT{c}")
                    ps = mps.tile([128, 512], F32, tag="mps")
                    for k in range(NK):
                        nc.tensor.matmul(
                            ps[:], wqP[:, k, :], xT_t[k][:, c * 512:(c + 1) * 512],
                            start=(k == 0), stop=(k == NK - 1),
                        )
                    nc.vector.tensor_scalar_add(qt[:], ps[:], bqk_t[:, P:P + 1])
                    QT.append(qt)
                KT = []
                for c in range(NC):
                    kt = qk.tile([128, 512], FP16, tag="qk", name=f"KT{c}")
                    ps = mps.tile([128, 512], F32, tag="mps")
                    for k in range(NK):
                        nc.tensor.matmul(
                            ps[:], wkP[:, k, :], zT_t[k][:, c * 512:(c + 1) * 512],
                            start=(k == 0), stop=(k == NK - 1),
                        )
                    nc.vector.tensor_scalar_add(kt[:], ps[:], bqk_t[:, 8 + P:9 + P])
                    KT.append(kt)

                # attention for the two heads of this pair
                for c in range(NC):
                    jlive = [j for j in range(NJ) if 128 * j <= 512 * c + 511]
                    yp = [yps.tile([65, 512], F32, tag="yps", name=f"yp{P}_{c}_{h01}")
                          for h01 in range(2)]
                    for j in jlive:
                        kband = j - 4 * c
                        x0 = 128 * max(kband, 0)
                        ktile = KT[j // 4]
                        jj = j % 4
                        sp = sps.tile([128, 1024], F32, tag="sps")
                        at = apool.tile([128, 1024], FP16, tag="at")
                        for h01 in range(2):
                            hoff = 64 * h01
                            nc.tensor.matmul(
                                sp[:, h01 * 512 + x0:(h01 + 1) * 512],
                                ktile[hoff:hoff + 64, jj * 128:(jj + 1) * 128],
                                QT[c][hoff:hoff + 64, x0:512],
                                start=True, stop=True,
                            )
                        # one exp over both heads' regions (strided 2-bank AP)
                        sp_v = sp[:].rearrange("p (h x) -> p h x", x=512)[:, :, x0:512]
                        at_v = at[:].rearrange("p (h x) -> p h x", x=512)[:, :, x0:512]
                        nc.scalar.activation(at_v, sp_v, Exp, bias=0.0, scale=SCALE)
                        if kband >= 0:
                            at_m = at[:].rearrange(
                                "p (h x) -> p h x", x=512)[:, :, x0:x0 + 128]
                            mk_m = maskt_t[:].rearrange("p (h x) -> p h x", x=128)
                            nc.vector.tensor_mul(at_m, at_m, mk_m)
                        for h01 in range(2):
                            h = 2 * P + h01
                            nc.tensor.matmul(
                                yp[h01][:, x0:512],
                                V_t[:, j, h * 65:(h + 1) * 65],
                                at[:, h01 * 512 + x0:(h01 + 1) * 512],
                                start=(j == jlive[0]), stop=(j == jlive[-1]),
                                skip_group_check=True,
                            )
                    # normalization + eviction to packed pair layout
                    for h01 in range(2):
                        se_t = norm.tile([1, 512], F32, tag="se")
                        nc.vector.tensor_copy(se_t[:], yp[h01][64:65, :])
                        r_t = norm.tile([1, 512], F32, tag="rt")
                        nc.vector.reciprocal_approx_fast(r_t[:], se_t[:])
                        bc_t = norm.tile([64, 512], F32, tag="bc")
                        nc.gpsimd.partition_broadcast(bc_t[:], r_t[:])
                        hoff = 64 * h01
                        nc.vector.tensor_mul(
                            yTt[c][P][hoff:hoff + 64, :], yp[h01][0:64, :], bc_t[:])
                    # keep the PE activity monitor warm through the
                    # end-of-chunk pipeline drain (no psum side effects)
                    for _ in range(3):
                        nc.tensor.ldweights(scratch_t[:, 0:128])

            # ---- output projection: out = yT_cat.T @ Wp + bp ----
            for dc in range(2):
                for m in range(NJ):
                    c, mm = m // 4, m % 4
                    opsum = mps if m % 2 == 0 else sps
                    ps = opsum.tile([128, 512], F32,
                                    tag="mps" if m % 2 == 0 else "sps")
                    for ht in range(NP):
                        nc.tensor.matmul(
                            ps[:], yTt[c][ht][:, mm * 128:(mm + 1) * 128],
                            wph[dc][:, ht, :],
                            start=(ht == 0), stop=(ht == NP - 1),
                        )
                    last = (dc == 1 and m == NJ - 1)
                    nsplit = 2 if last else 1
                    w = 512 // nsplit
                    for sp_i in range(nsplit):
                        o_t = opool.tile([128, 512], FP16, tag="ot")
                        nc.vector.tensor_add(
                            o_t[:, 0:w], ps[:, sp_i * w:(sp_i + 1) * w],
                            bpb_t[:, dc * 512 + sp_i * w:dc * 512 + (sp_i + 1) * w])
                        nc.sync.dma_start(
                            out_d.ap()[:, m, dc * 512 + sp_i * w:dc * 512 + (sp_i + 1) * w],
                            o_t[:, 0:w])

    nc.compile()
    return nc


_CACHED_NC = None


def _get_program():
    global _CACHED_NC
    if _CACHED_NC is None:
        _CACHED_NC = build_program()
    return _CACHED_NC


def _prep_shared(Wq, bq, Wk, bk, Wv, bv, Wp, bp, mask):
    assert np.array_equal(
        np.asarray(mask), np.tril(np.ones((T, T), dtype=bool))
    ), "kernel specialized for causal (tril) mask"
    # [d, he] flat weights
    wq_cat = np.asarray(Wq, np.float32).transpose(1, 0, 2).reshape(D, H * E)
    wk_cat = np.asarray(Wk, np.float32).transpose(1, 0, 2).reshape(D, H * E)
    wv_cat = np.asarray(Wv, np.float32).transpose(1, 0, 2).reshape(D, H * E)
    wp_cat = np.asarray(Wp, np.float32)
    # partition-major stagings: every DMA line is contiguous per partition
    wq = np.ascontiguousarray(
        wq_cat.reshape(NK, 128, NP, 128).transpose(1, 2, 0, 3).reshape(128, -1)
    ).astype(np.float16)
    wk = np.ascontiguousarray(
        wk_cat.reshape(NK, 128, NP, 128).transpose(1, 2, 0, 3).reshape(128, -1)
    ).astype(np.float16)
    wv = np.ascontiguousarray(
        wv_cat.reshape(NK, 128, 2, 512).transpose(1, 2, 0, 3).reshape(128, -1)
    ).astype(np.float16)
    wp = np.ascontiguousarray(
        wp_cat.reshape(NK, 128, 2, 512).transpose(1, 2, 0, 3).reshape(128, -1)
    ).astype(np.float16)
    bq_c = np.asarray(bq, np.float32).reshape(-1)
    bk_c = np.asarray(bk, np.float32).reshape(-1)
    bqk = np.concatenate(
        [bq_c.reshape(8, 128).T, bk_c.reshape(8, 128).T], axis=1
    ).astype(np.float32)
    tri = np.triu(np.ones((128, 128), np.float16))  # allow z <= x
    maskt = np.concatenate([tri, tri], axis=1)      # [128, 256] for both heads
    bvb = np.ascontiguousarray(np.broadcast_to(
        np.asarray(bv, np.float32).reshape(1, -1), (128, H * E)).astype(np.float16))
    bpb = np.ascontiguousarray(np.broadcast_to(
        np.asarray(bp, np.float32).reshape(1, -1), (128, H * E)).astype(np.float32))
    return {
        "wq": wq, "wk": wk, "wv": wv, "wp": wp,
        "bqk": np.ascontiguousarray(bqk),
        "bvb": bvb, "bpb": bpb,
        "maskt": np.ascontiguousarray(maskt),
    }


def kernel(x, z, Wq, bq, Wk, bk, Wv, bv, Wp, bp, mask, _trace=False, _trace_kwargs=None):
    x = np.asarray(x, np.float32)
    z = np.asarray(z, np.float32)
    shared = _prep_shared(Wq, bq, Wk, bk, Wv, bv, Wp, bp, mask)
    in_maps = []
    for b in range(B):
        m = dict(shared)
        m["xT"] = np.ascontiguousarray(x[b].T.astype(np.float16))
        zt = z[b].T.astype(np.float16)
        m["zT"] = np.ascontiguousarray(zt)
        m["zTzb"] = np.ascontiguousarray(
            zt.reshape(NK, 128, NJ, 128).transpose(1, 2, 0, 3).reshape(128, -1))
        in_maps.append(m)
    nc = _get_program()
    res = run_bass_kernel_spmd(
        nc, in_maps, core_ids=list(range(B)),
        trace=_trace, **(_trace_kwargs or {}),
    )
    # out staging is [p, m, d]: row m*128+p of the logical [T, D] output
    out = np.stack([
        np.asarray(r["out"]).transpose(1, 0, 2).reshape(T, D) for r in res.results
    ]).astype(np.float32)
    if _trace:
        kernel.last_results = res
    return out
